# revision 6
# baseline (speedup 1.0000x reference)
"""Trainium2 Bass kernel for the additive-attention model.

Math (per batch b):
    res_q = query @ Wq.T                  [H]
    res_c = ctx @ Wc.T + bc               [L, H]
    tmp   = tanh(res_q + res_c)           [L, H]
    logit = tmp @ Wo.T + bo               [L, 1]
    wexp  = mask * exp(logit)             [L, 1]
    w     = wexp / (sum_L wexp + 1e-5)    [L, 1]
    out   = sum_L w * ctx                 [C]

Sharding: data-parallel over batch B=32 across 8 cores (4 batches/core),
weights replicated.  Inside a core everything is computed in a transposed
layout (H on partitions) so the res_q/bc bias folds into the tanh
activation and the Wo contraction is a matmul.
"""

import sys

if "/opt/trn_rl_repo" not in sys.path:
    sys.path.insert(0, "/opt/trn_rl_repo")

from contextlib import ExitStack

import numpy as np

import concourse.bacc as bacc
import concourse.bass as bass
import concourse.tile as tile
from concourse import mybir
from concourse.bass_utils import run_bass_kernel_spmd
from concourse.masks import make_identity

F32 = mybir.dt.float32
AF = mybir.ActivationFunctionType

B, L, C, Q, H = 32, 2048, 512, 512, 1024
NCORES = 8
BLOC = B // NCORES  # 4 batches per core
LCHUNK = 512
NCHUNKS = L // LCHUNK  # 4
NSUB = LCHUNK // 128  # 4 L-subtiles per chunk
HM = H // 128  # 8 H tiles (output partition dim of mm1)
KC = C // 128  # 4 contraction tiles over C
KQ = Q // 128  # 4 contraction tiles over Q


def build_nc() -> bass.Bass:
    nc = bacc.Bacc()

    ctx_d = nc.dram_tensor("ctx", (BLOC, L, C), F32, kind="ExternalInput")
    qT_d = nc.dram_tensor("qT", (Q, BLOC), F32, kind="ExternalInput")
    mask_d = nc.dram_tensor("mask", (BLOC, L), F32, kind="ExternalInput")
    wcT_d = nc.dram_tensor("wcT", (C, H), F32, kind="ExternalInput")
    wqT_d = nc.dram_tensor("wqT", (Q, H), F32, kind="ExternalInput")
    wo_d = nc.dram_tensor("wo", (128, HM), F32, kind="ExternalInput")
    bc_d = nc.dram_tensor("bc", (1, H), F32, kind="ExternalInput")
    bo_d = nc.dram_tensor("bo", (1, 1), F32, kind="ExternalInput")

    out_d = nc.dram_tensor("out", (BLOC, C), F32, kind="ExternalOutput")
    w_d = nc.dram_tensor("w", (BLOC, L), F32, kind="ExternalOutput")

    with ExitStack() as ctx_es:
        tc = ctx_es.enter_context(tile.TileContext(nc))
        const = ctx_es.enter_context(tc.tile_pool(name="const", bufs=1))
        ctxp = ctx_es.enter_context(tc.tile_pool(name="ctxp", bufs=3))
        ctxTp = ctx_es.enter_context(tc.tile_pool(name="ctxTp", bufs=2))
        tmpp = ctx_es.enter_context(tc.tile_pool(name="tmpp", bufs=2))
        wrowp = ctx_es.enter_context(tc.tile_pool(name="wrowp", bufs=2))
        smallp = ctx_es.enter_context(tc.tile_pool(name="smallp", bufs=4))
        ps_tr = ctx_es.enter_context(tc.tile_pool(name="ps_tr", bufs=2, space="PSUM"))
        ps_mm1 = ctx_es.enter_context(tc.tile_pool(name="ps_mm1", bufs=2, space="PSUM"))
        ps_mm2 = ctx_es.enter_context(tc.tile_pool(name="ps_mm2", bufs=1, space="PSUM"))
        ps_wt = ctx_es.enter_context(tc.tile_pool(name="ps_wt", bufs=1, space="PSUM"))
        ps_out = ctx_es.enter_context(tc.tile_pool(name="ps_out", bufs=1, space="PSUM"))

        # ---- constants ----
        ident = const.tile([128, 128], F32)
        make_identity(nc, ident)

        wcT_sb = const.tile([128, KC, H], F32)
        nc.sync.dma_start(wcT_sb, wcT_d[:].rearrange("(k p) h -> p k h", p=128))
        wqT_sb = const.tile([128, KQ, H], F32)
        nc.sync.dma_start(wqT_sb, wqT_d[:].rearrange("(k p) h -> p k h", p=128))
        wo_sb = const.tile([128, HM], F32)
        nc.sync.dma_start(wo_sb, wo_d[:])
        bc_sb = const.tile([1, H], F32)
        nc.sync.dma_start(bc_sb, bc_d[:])
        bo_sb = const.tile([1, 1], F32)
        nc.sync.dma_start(bo_sb, bo_d[:])
        # engine operands must start at partition 0, so keep the mask flat on one partition
        mask_sb = const.tile([1, BLOC * L], F32)
        nc.sync.dma_start(mask_sb, mask_d[:].rearrange("b l -> (b l)")[None, :])
        qT_sb = const.tile([128, KQ, BLOC], F32)
        nc.sync.dma_start(qT_sb, qT_d[:].rearrange("(k p) b -> p k b", p=128))
        ones_sb = const.tile([1, BLOC], F32)
        nc.vector.memset(ones_sb, 1.0)

        # ---- res_q + bc, transposed: rq_sb[p, m*BLOC + b] = (q_b @ Wq.T + bc)[m*128+p]
        ps_rq = ps_mm2.tile([128, HM * BLOC], F32, tag="rq")
        for m in range(HM):
            o = ps_rq[:, m * BLOC : (m + 1) * BLOC]
            for k in range(KQ):
                nc.tensor.matmul(
                    o,
                    lhsT=wqT_sb[:, k, m * 128 : (m + 1) * 128],
                    rhs=qT_sb[:, k, :],
                    start=(k == 0),
                    stop=False,
                )
            # + bc via rank-1 (K=1) matmul: bc_chunk.T @ ones
            nc.tensor.matmul(
                o,
                lhsT=bc_sb[0:1, m * 128 : (m + 1) * 128],
                rhs=ones_sb[0:1, :],
                start=False,
                stop=True,
            )
        rq_sb = const.tile([128, HM * BLOC], F32)
        nc.vector.tensor_copy(rq_sb, ps_rq)

        # ---- main loop ----
        for b in range(BLOC):
            wrow_sb = wrowp.tile([1, L], F32, tag="wrow")
            ps_o = ps_out.tile([1, C], F32, tag="out")
            for c in range(NCHUNKS):
                # load ctx chunk [128, NSUB, C]; subtile i holds rows 512c+128i..+128
                ctx_sb = ctxp.tile([128, NSUB, C], F32, tag="ctx")
                nc.sync.dma_start(
                    ctx_sb,
                    ctx_d[b, c * LCHUNK : (c + 1) * LCHUNK, :].rearrange(
                        "(i p) c -> p i c", p=128
                    ),
                )

                # transpose to [C-part, L] layout (PE transposes, then copy psum->sbuf)
                ctxT_sb = ctxTp.tile([128, KC, LCHUNK], F32, tag="ctxT")
                for j in range(KC):
                    ps_t = ps_tr.tile([128, LCHUNK], F32, tag="tr")
                    for i in range(NSUB):
                        nc.tensor.transpose(
                            ps_t[:, i * 128 : (i + 1) * 128],
                            ctx_sb[:, i, j * 128 : (j + 1) * 128],
                            ident,
                        )
                    nc.vector.tensor_copy(ctxT_sb[:, j, :], ps_t)

                # mm1: res_c.T tiles [128H, 512L]; tanh(+rq bias) -> tmpT
                tmpT_sb = tmpp.tile([128, HM, LCHUNK], F32, tag="tmpT")
                for m in range(HM):
                    ps_1 = ps_mm1.tile([128, LCHUNK], F32, tag="mm1")
                    for j in range(KC):
                        nc.tensor.matmul(
                            ps_1,
                            lhsT=wcT_sb[:, j, m * 128 : (m + 1) * 128],
                            rhs=ctxT_sb[:, j, :],
                            start=(j == 0),
                            stop=(j == KC - 1),
                        )
                    nc.scalar.activation(
                        tmpT_sb[:, m, :],
                        ps_1,
                        AF.Tanh,
                        bias=rq_sb[:, m * BLOC + b : m * BLOC + b + 1],
                        scale=1.0,
                    )

                # mm2: logits [1, 512] = sum_m wo_m.T @ tmpT_m
                ps_2 = ps_mm2.tile([1, LCHUNK], F32, tag="mm2")
                for m in range(HM):
                    nc.tensor.matmul(
                        ps_2,
                        lhsT=wo_sb[:, m : m + 1],
                        rhs=tmpT_sb[:, m, :],
                        start=(m == 0),
                        stop=(m == HM - 1),
                    )

                # wexp = mask * exp(logit + bo)
                wseg = wrow_sb[0:1, c * LCHUNK : (c + 1) * LCHUNK]
                nc.scalar.activation(wseg, ps_2, AF.Exp, bias=bo_sb[0:1, 0:1], scale=1.0)
                nc.vector.tensor_mul(
                    wseg,
                    wseg,
                    mask_sb[0:1, b * L + c * LCHUNK : b * L + (c + 1) * LCHUNK],
                )

                # transpose wexp back to [L-part, 1] via K=1 matmuls with ones
                ps_w = ps_wt.tile([128, NSUB], F32, tag="wt")
                for i in range(NSUB):
                    nc.tensor.matmul(
                        ps_w[:, i : i + 1],
                        lhsT=wrow_sb[0:1, c * LCHUNK + i * 128 : c * LCHUNK + (i + 1) * 128],
                        rhs=ones_sb[0:1, 0:1],
                        start=True,
                        stop=True,
                    )
                wT_sb = smallp.tile([128, NSUB], F32, tag="wT")
                nc.vector.tensor_copy(wT_sb, ps_w)

                # mm3: accumulate unnormalized output over the whole batch
                for i in range(NSUB):
                    nc.tensor.matmul(
                        ps_o,
                        lhsT=wT_sb[:, i : i + 1],
                        rhs=ctx_sb[:, i, :],
                        start=(c == 0 and i == 0),
                        stop=(c == NCHUNKS - 1 and i == NSUB - 1),
                    )

            # ---- batch epilogue: normalize ----
            s_sb = smallp.tile([1, 1], F32, tag="s")
            nc.vector.tensor_reduce(
                s_sb, wrow_sb, axis=mybir.AxisListType.X, op=mybir.AluOpType.add
            )
            nc.vector.tensor_scalar_add(s_sb, s_sb, 1e-5)
            r_sb = smallp.tile([1, 1], F32, tag="r")
            nc.vector.reciprocal(r_sb, s_sb)

            wout_sb = wrowp.tile([1, L], F32, tag="wout")
            nc.vector.tensor_scalar_mul(wout_sb, wrow_sb, r_sb[0:1, 0:1])
            nc.sync.dma_start(w_d[b : b + 1, :], wout_sb)

            o_sb = smallp.tile([1, C], F32, tag="o")
            nc.vector.tensor_scalar_mul(o_sb, ps_o, r_sb[0:1, 0:1])
            nc.sync.dma_start(out_d[b : b + 1, :], o_sb)

    nc.compile()
    return nc


_NC_CACHE: list = []


def _get_nc() -> bass.Bass:
    if not _NC_CACHE:
        _NC_CACHE.append(build_nc())
    return _NC_CACHE[0]


def kernel(query, context, mapping_mask, Wq, Wc, bc, Wo, bo):
    query = np.asarray(query, dtype=np.float32)
    context = np.asarray(context, dtype=np.float32)
    mapping_mask = np.asarray(mapping_mask, dtype=np.float32)
    Wq = np.asarray(Wq, dtype=np.float32)
    Wc = np.asarray(Wc, dtype=np.float32)
    bc = np.asarray(bc, dtype=np.float32)
    Wo = np.asarray(Wo, dtype=np.float32)
    bo = np.asarray(bo, dtype=np.float32)

    wcT = np.ascontiguousarray(Wc.T)
    wqT = np.ascontiguousarray(Wq.T)
    wo = np.ascontiguousarray(Wo.reshape(HM, 128).T)
    bc2 = np.ascontiguousarray(bc.reshape(1, H))
    bo2 = np.ascontiguousarray(bo.reshape(1, 1))

    in_maps = []
    for core in range(NCORES):
        sl = slice(core * BLOC, (core + 1) * BLOC)
        in_maps.append(
            {
                "ctx": np.ascontiguousarray(context[sl]),
                "qT": np.ascontiguousarray(query[sl].T),
                "mask": np.ascontiguousarray(mapping_mask[sl, :, 0]),
                "wcT": wcT,
                "wqT": wqT,
                "wo": wo,
                "bc": bc2,
                "bo": bo2,
            }
        )

    res = run_bass_kernel_spmd(_get_nc(), in_maps, core_ids=list(range(NCORES)))
    outs = res.results
    output = np.concatenate([o["out"] for o in outs], axis=0)
    weights = np.concatenate([o["w"] for o in outs], axis=0).reshape(B, L, 1)
    return output, weights


# revision 10
# speedup vs baseline: 2.7140x; 2.7140x over previous
"""Trainium2 Bass kernel for the additive-attention model.

Math (per batch b):
    res_q = query @ Wq.T                  [H]
    res_c = ctx @ Wc.T + bc               [L, H]
    tmp   = tanh(res_q + res_c)           [L, H]
    logit = tmp @ Wo.T + bo               [L, 1]
    wexp  = mask * exp(logit)             [L, 1]
    w     = wexp / (sum_L wexp + 1e-5)    [L, 1]
    out   = sum_L w * ctx                 [C]

Sharding: data-parallel over batch B=32 across 8 cores (4 batches/core),
weights replicated.  Inside a core everything is computed in a transposed
layout (H on partitions) so the res_q/bc bias folds into the tanh
activation and the Wo contraction is a matmul.  The two inner matmuls
(res_c and the Wo contraction) run in bf16 — fp32 matmul on trn2 costs
two PE passes (hi/lo), bf16 one.  The output contraction stays fp32.
"""

import sys

if "/opt/trn_rl_repo" not in sys.path:
    sys.path.insert(0, "/opt/trn_rl_repo")

from contextlib import ExitStack

import ml_dtypes
import numpy as np

import concourse.bacc as bacc
import concourse.bass as bass
import concourse.tile as tile
from concourse import mybir
from concourse.bass_utils import run_bass_kernel_spmd

F32 = mybir.dt.float32
BF16 = mybir.dt.bfloat16
AF = mybir.ActivationFunctionType

B, L, C, Q, H = 32, 2048, 512, 512, 1024
NCORES = 8
BLOC = B // NCORES  # 4 batches per core
LCHUNK = 512
NCHUNKS = L // LCHUNK  # 4
NSUB = LCHUNK // 128  # 4 L-subtiles per chunk
HM = H // 128  # 8 H tiles (output partition dim of mm1)
KC = C // 128  # 4 contraction tiles over C
KQ = Q // 128  # 4 contraction tiles over Q


def build_nc() -> bass.Bass:
    nc = bacc.Bacc()

    ctx_d = nc.dram_tensor("ctx", (BLOC, L, C), F32, kind="ExternalInput")
    ctxT_d = nc.dram_tensor("ctxT", (BLOC, C, L), BF16, kind="ExternalInput")
    qT_d = nc.dram_tensor("qT", (Q, BLOC), F32, kind="ExternalInput")
    mask_d = nc.dram_tensor("mask", (BLOC, L), F32, kind="ExternalInput")
    wcT_d = nc.dram_tensor("wcT", (C, H), BF16, kind="ExternalInput")
    wqT_d = nc.dram_tensor("wqT", (Q, H), F32, kind="ExternalInput")
    wo_d = nc.dram_tensor("wo", (128, HM), BF16, kind="ExternalInput")
    bc_d = nc.dram_tensor("bc", (1, H), F32, kind="ExternalInput")
    bo_d = nc.dram_tensor("bo", (1, 1), F32, kind="ExternalInput")

    out_d = nc.dram_tensor("out", (BLOC, C), F32, kind="ExternalOutput")
    w_d = nc.dram_tensor("w", (BLOC, L), F32, kind="ExternalOutput")

    with ExitStack() as ctx_es:
        tc = ctx_es.enter_context(tile.TileContext(nc))
        const = ctx_es.enter_context(tc.tile_pool(name="const", bufs=1))
        ctxp = ctx_es.enter_context(tc.tile_pool(name="ctxp", bufs=3))
        ctxTp = ctx_es.enter_context(tc.tile_pool(name="ctxTp", bufs=3))
        tmpp = ctx_es.enter_context(tc.tile_pool(name="tmpp", bufs=2))
        wrowp = ctx_es.enter_context(tc.tile_pool(name="wrowp", bufs=2))
        smallp = ctx_es.enter_context(tc.tile_pool(name="smallp", bufs=4))
        ps_mm1 = ctx_es.enter_context(tc.tile_pool(name="ps_mm1", bufs=3, space="PSUM"))
        ps_mm2 = ctx_es.enter_context(tc.tile_pool(name="ps_mm2", bufs=1, space="PSUM"))
        ps_wt = ctx_es.enter_context(tc.tile_pool(name="ps_wt", bufs=1, space="PSUM"))
        ps_out = ctx_es.enter_context(tc.tile_pool(name="ps_out", bufs=2, space="PSUM"))

        # ---- constants ----
        wcT_sb = const.tile([128, KC, H], BF16)
        nc.sync.dma_start(wcT_sb, wcT_d[:].rearrange("(k p) h -> p k h", p=128))
        wqT_sb = const.tile([128, KQ, H], F32)
        nc.sync.dma_start(wqT_sb, wqT_d[:].rearrange("(k p) h -> p k h", p=128))
        wo_sb = const.tile([128, HM], BF16)
        nc.sync.dma_start(wo_sb, wo_d[:])
        bc_sb = const.tile([1, H], F32)
        nc.sync.dma_start(bc_sb, bc_d[:])
        bo_sb = const.tile([1, 1], F32)
        nc.sync.dma_start(bo_sb, bo_d[:])
        # engine operands must start at partition 0, so keep the mask flat on one partition
        mask_sb = const.tile([1, BLOC * L], F32)
        nc.sync.dma_start(mask_sb, mask_d[:].rearrange("b l -> (b l)")[None, :])
        qT_sb = const.tile([128, KQ, BLOC], F32)
        nc.sync.dma_start(qT_sb, qT_d[:].rearrange("(k p) b -> p k b", p=128))
        ones_sb = const.tile([1, BLOC], F32)
        nc.vector.memset(ones_sb, 1.0)

        # ---- res_q + bc, transposed: rq_sb[p, m*BLOC + b] = (q_b @ Wq.T + bc)[m*128+p]
        ps_rq = ps_mm2.tile([128, HM * BLOC], F32, tag="rq")
        for m in range(HM):
            o = ps_rq[:, m * BLOC : (m + 1) * BLOC]
            for k in range(KQ):
                nc.tensor.matmul(
                    o,
                    lhsT=wqT_sb[:, k, m * 128 : (m + 1) * 128],
                    rhs=qT_sb[:, k, :],
                    start=(k == 0),
                    stop=False,
                )
            # + bc via rank-1 (K=1) matmul: bc_chunk.T @ ones
            nc.tensor.matmul(
                o,
                lhsT=bc_sb[0:1, m * 128 : (m + 1) * 128],
                rhs=ones_sb[0:1, :],
                start=False,
                stop=True,
            )
        rq_sb = const.tile([128, HM * BLOC], F32)
        nc.vector.tensor_copy(rq_sb, ps_rq)

        # ---- main loop ----
        for b in range(BLOC):
            wrow_sb = wrowp.tile([1, L], F32, tag="wrow")
            ps_o = ps_out.tile([1, C], F32, tag="out")
            for c in range(NCHUNKS):
                # ctx chunk in natural layout (fp32, for the output matmul)
                ctx_sb = ctxp.tile([128, NSUB, C], F32, tag="ctx")
                nc.sync.dma_start(
                    ctx_sb,
                    ctx_d[b, c * LCHUNK : (c + 1) * LCHUNK, :].rearrange(
                        "(i p) c -> p i c", p=128
                    ),
                )
                # ctx chunk transposed (bf16, rhs of the res_c matmul)
                ctxT_sb = ctxTp.tile([128, KC, LCHUNK], BF16, tag="ctxT")
                nc.sync.dma_start(
                    ctxT_sb,
                    ctxT_d[b, :, c * LCHUNK : (c + 1) * LCHUNK].rearrange(
                        "(j p) l -> p j l", p=128
                    ),
                )

                # mm1: res_c.T tiles [128H, 512L]; tanh(+rq bias) -> tmpT (bf16)
                tmpT_sb = tmpp.tile([128, HM, LCHUNK], BF16, tag="tmpT")
                for m in range(HM):
                    ps_1 = ps_mm1.tile([128, LCHUNK], F32, tag="mm1")
                    for j in range(KC):
                        nc.tensor.matmul(
                            ps_1,
                            lhsT=wcT_sb[:, j, m * 128 : (m + 1) * 128],
                            rhs=ctxT_sb[:, j, :],
                            start=(j == 0),
                            stop=(j == KC - 1),
                        )
                    nc.scalar.activation(
                        tmpT_sb[:, m, :],
                        ps_1,
                        AF.Tanh,
                        bias=rq_sb[:, m * BLOC + b : m * BLOC + b + 1],
                        scale=1.0,
                    )

                # mm2: logits [1, 512] = sum_m wo_m.T @ tmpT_m
                ps_2 = ps_mm2.tile([1, LCHUNK], F32, tag="mm2")
                for m in range(HM):
                    nc.tensor.matmul(
                        ps_2,
                        lhsT=wo_sb[:, m : m + 1],
                        rhs=tmpT_sb[:, m, :],
                        start=(m == 0),
                        stop=(m == HM - 1),
                    )

                # wexp = mask * exp(logit + bo)
                wseg = wrow_sb[0:1, c * LCHUNK : (c + 1) * LCHUNK]
                nc.scalar.activation(wseg, ps_2, AF.Exp, bias=bo_sb[0:1, 0:1], scale=1.0)
                nc.vector.tensor_mul(
                    wseg,
                    wseg,
                    mask_sb[0:1, b * L + c * LCHUNK : b * L + (c + 1) * LCHUNK],
                )

                # transpose wexp back to [L-part, 1] via K=1 matmuls with ones
                ps_w = ps_wt.tile([128, NSUB], F32, tag="wt")
                for i in range(NSUB):
                    nc.tensor.matmul(
                        ps_w[:, i : i + 1],
                        lhsT=wrow_sb[0:1, c * LCHUNK + i * 128 : c * LCHUNK + (i + 1) * 128],
                        rhs=ones_sb[0:1, 0:1],
                        start=True,
                        stop=True,
                    )
                wT_sb = smallp.tile([128, NSUB], F32, tag="wT")
                nc.vector.tensor_copy(wT_sb, ps_w)

                # mm3: accumulate unnormalized output over the whole batch (fp32)
                for i in range(NSUB):
                    nc.tensor.matmul(
                        ps_o,
                        lhsT=wT_sb[:, i : i + 1],
                        rhs=ctx_sb[:, i, :],
                        start=(c == 0 and i == 0),
                        stop=(c == NCHUNKS - 1 and i == NSUB - 1),
                    )

            # ---- batch epilogue: normalize ----
            s_sb = smallp.tile([1, 1], F32, tag="s")
            nc.vector.tensor_reduce(
                s_sb, wrow_sb, axis=mybir.AxisListType.X, op=mybir.AluOpType.add
            )
            nc.vector.tensor_scalar_add(s_sb, s_sb, 1e-5)
            r_sb = smallp.tile([1, 1], F32, tag="r")
            nc.vector.reciprocal(r_sb, s_sb)

            wout_sb = wrowp.tile([1, L], F32, tag="wout")
            nc.vector.tensor_scalar_mul(wout_sb, wrow_sb, r_sb[0:1, 0:1])
            nc.sync.dma_start(w_d[b : b + 1, :], wout_sb)

            o_sb = smallp.tile([1, C], F32, tag="o")
            nc.vector.tensor_scalar_mul(o_sb, ps_o, r_sb[0:1, 0:1])
            nc.sync.dma_start(out_d[b : b + 1, :], o_sb)

    nc.compile()
    return nc


_NC_CACHE: list = []


def _get_nc() -> bass.Bass:
    if not _NC_CACHE:
        _NC_CACHE.append(build_nc())
    return _NC_CACHE[0]


def kernel(query, context, mapping_mask, Wq, Wc, bc, Wo, bo):
    query = np.asarray(query, dtype=np.float32)
    context = np.asarray(context, dtype=np.float32)
    mapping_mask = np.asarray(mapping_mask, dtype=np.float32)
    Wq = np.asarray(Wq, dtype=np.float32)
    Wc = np.asarray(Wc, dtype=np.float32)
    bc = np.asarray(bc, dtype=np.float32)
    Wo = np.asarray(Wo, dtype=np.float32)
    bo = np.asarray(bo, dtype=np.float32)

    wcT = np.ascontiguousarray(Wc.T).astype(ml_dtypes.bfloat16)
    wqT = np.ascontiguousarray(Wq.T)
    wo = np.ascontiguousarray(Wo.reshape(HM, 128).T).astype(ml_dtypes.bfloat16)
    bc2 = np.ascontiguousarray(bc.reshape(1, H))
    bo2 = np.ascontiguousarray(bo.reshape(1, 1))
    ctxT_all = np.ascontiguousarray(context.transpose(0, 2, 1)).astype(
        ml_dtypes.bfloat16
    )

    in_maps = []
    for core in range(NCORES):
        sl = slice(core * BLOC, (core + 1) * BLOC)
        in_maps.append(
            {
                "ctx": np.ascontiguousarray(context[sl]),
                "ctxT": np.ascontiguousarray(ctxT_all[sl]),
                "qT": np.ascontiguousarray(query[sl].T),
                "mask": np.ascontiguousarray(mapping_mask[sl, :, 0]),
                "wcT": wcT,
                "wqT": wqT,
                "wo": wo,
                "bc": bc2,
                "bo": bo2,
            }
        )

    res = run_bass_kernel_spmd(_get_nc(), in_maps, core_ids=list(range(NCORES)))
    outs = res.results
    output = np.concatenate([o["out"] for o in outs], axis=0)
    weights = np.concatenate([o["w"] for o in outs], axis=0).reshape(B, L, 1)
    return output, weights


# revision 14
# speedup vs baseline: 3.4280x; 1.2631x over previous
"""Trainium2 Bass kernel for the additive-attention model.

Math (per batch b):
    res_q = query @ Wq.T                  [H]
    res_c = ctx @ Wc.T + bc               [L, H]
    tmp   = tanh(res_q + res_c)           [L, H]
    logit = tmp @ Wo.T + bo               [L, 1]
    wexp  = mask * exp(logit)             [L, 1]
    w     = wexp / (sum_L wexp + 1e-5)    [L, 1]
    out   = sum_L w * ctx                 [C]

Sharding: data-parallel over batch B=32 across 8 cores (4 batches/core),
weights replicated.  Everything is computed in a transposed layout
(H on partitions) so the res_q/bc bias folds into the tanh activation
and the Wo contraction is a matmul.  Matmul operands are fp16 (fp32
matmul on trn2 costs two PE passes, 16-bit one; fp16 keeps 10 mantissa
bits), with all accumulation/normalization in fp32.
"""

import sys

if "/opt/trn_rl_repo" not in sys.path:
    sys.path.insert(0, "/opt/trn_rl_repo")

from contextlib import ExitStack

import numpy as np

import concourse.bacc as bacc
import concourse.bass as bass
import concourse.tile as tile
from concourse import mybir
from concourse.bass_utils import run_bass_kernel_spmd

F32 = mybir.dt.float32
F16 = mybir.dt.float16
AF = mybir.ActivationFunctionType

B, L, C, Q, H = 32, 2048, 512, 512, 1024
NCORES = 8
BLOC = B // NCORES  # 4 batches per core
LCHUNK = 512
NCHUNKS = L // LCHUNK  # 4
NSUB = LCHUNK // 128  # 4 L-subtiles per chunk
HM = H // 128  # 8 H tiles (output partition dim of mm1)
KC = C // 128  # 4 contraction tiles over C
KQ = Q // 128  # 4 contraction tiles over Q


def build_nc() -> bass.Bass:
    nc = bacc.Bacc()

    ctx_d = nc.dram_tensor("ctx", (BLOC, L, C), F16, kind="ExternalInput")
    ctxT_d = nc.dram_tensor("ctxT", (BLOC, C, L), F16, kind="ExternalInput")
    qT_d = nc.dram_tensor("qT", (Q, BLOC), F16, kind="ExternalInput")
    mask_d = nc.dram_tensor("mask", (BLOC, L), F32, kind="ExternalInput")
    wcT_d = nc.dram_tensor("wcT", (C, H), F16, kind="ExternalInput")
    wqT_d = nc.dram_tensor("wqT", (Q, H), F16, kind="ExternalInput")
    wo_d = nc.dram_tensor("wo", (128, HM), F16, kind="ExternalInput")
    bc_d = nc.dram_tensor("bc", (1, H), F32, kind="ExternalInput")
    bo_d = nc.dram_tensor("bo", (1, 1), F32, kind="ExternalInput")

    out_d = nc.dram_tensor("out", (BLOC, C), F32, kind="ExternalOutput")
    w_d = nc.dram_tensor("w", (BLOC, L), F32, kind="ExternalOutput")

    with ExitStack() as ctx_es:
        tc = ctx_es.enter_context(tile.TileContext(nc))
        const = ctx_es.enter_context(tc.tile_pool(name="const", bufs=1))
        ctxp = ctx_es.enter_context(tc.tile_pool(name="ctxp", bufs=3))
        ctxTp = ctx_es.enter_context(tc.tile_pool(name="ctxTp", bufs=3))
        tmpp = ctx_es.enter_context(tc.tile_pool(name="tmpp", bufs=2))
        wrowp = ctx_es.enter_context(tc.tile_pool(name="wrowp", bufs=2))
        smallp = ctx_es.enter_context(tc.tile_pool(name="smallp", bufs=4))
        ps_mm1 = ctx_es.enter_context(tc.tile_pool(name="ps_mm1", bufs=2, space="PSUM"))
        ps_mm2 = ctx_es.enter_context(tc.tile_pool(name="ps_mm2", bufs=2, space="PSUM"))
        ps_wt = ctx_es.enter_context(tc.tile_pool(name="ps_wt", bufs=2, space="PSUM"))
        ps_out = ctx_es.enter_context(tc.tile_pool(name="ps_out", bufs=1, space="PSUM"))

        # ---- constants (wcT first: the first matmuls need it) ----
        wcT_sb = const.tile([128, KC, H], F16)
        nc.sync.dma_start(wcT_sb, wcT_d[:].rearrange("(k p) h -> p k h", p=128))
        wqT_sb = const.tile([128, KQ, H], F16)
        nc.sync.dma_start(wqT_sb, wqT_d[:].rearrange("(k p) h -> p k h", p=128))
        qT_sb = const.tile([128, KQ, BLOC], F16)
        nc.sync.dma_start(qT_sb, qT_d[:].rearrange("(k p) b -> p k b", p=128))
        wo_sb = const.tile([128, HM], F16)
        nc.sync.dma_start(wo_sb, wo_d[:])
        bc_sb = const.tile([1, H], F32)
        nc.sync.dma_start(bc_sb, bc_d[:])
        bo_sb = const.tile([1, 1], F32)
        nc.sync.dma_start(bo_sb, bo_d[:])
        # engine operands must start at partition 0, so keep the mask flat on one partition
        mask_sb = const.tile([1, BLOC * L], F32)
        nc.sync.dma_start(mask_sb, mask_d[:].rearrange("b l -> (b l)")[None, :])
        onesf_sb = const.tile([1, BLOC], F32)
        nc.vector.memset(onesf_sb, 1.0)

        # ---- res_q + bc, transposed: rq_sb[p, m*BLOC + b] = (q_b @ Wq.T + bc)[m*128+p]
        ps_rq = ps_out.tile([128, HM * BLOC], F32, tag="rq")
        for m in range(HM):
            o = ps_rq[:, m * BLOC : (m + 1) * BLOC]
            for k in range(KQ):
                nc.tensor.matmul(
                    o,
                    lhsT=wqT_sb[:, k, m * 128 : (m + 1) * 128],
                    rhs=qT_sb[:, k, :],
                    start=(k == 0),
                    stop=False,
                )
            # + bc via rank-1 (K=1, fp32) matmul: bc_chunk.T @ ones
            nc.tensor.matmul(
                o,
                lhsT=bc_sb[0:1, m * 128 : (m + 1) * 128],
                rhs=onesf_sb[0:1, :],
                start=False,
                stop=True,
            )
        rq_sb = const.tile([128, HM * BLOC], F32)
        nc.vector.tensor_copy(rq_sb, ps_rq)

        # ---- main loop ----
        for b in range(BLOC):
            wrow_sb = wrowp.tile([1, L], F32, tag="wrow")
            sparts_sb = smallp.tile([1, NCHUNKS], F32, tag="sparts")
            ps_o = ps_out.tile([1, C], F32, tag="out")
            for c in range(NCHUNKS):
                # ctx chunk transposed (fp16, rhs of the res_c matmul) -- load
                # first, mm1 consumes it immediately
                ctxT_sb = ctxTp.tile([128, KC, LCHUNK], F16, tag="ctxT")
                nc.sync.dma_start(
                    ctxT_sb,
                    ctxT_d[b, :, c * LCHUNK : (c + 1) * LCHUNK].rearrange(
                        "(j p) l -> p j l", p=128
                    ),
                )
                # ctx chunk in natural layout (fp16, for the output matmul)
                ctx_sb = ctxp.tile([128, NSUB, C], F16, tag="ctx")
                nc.sync.dma_start(
                    ctx_sb,
                    ctx_d[b, c * LCHUNK : (c + 1) * LCHUNK, :].rearrange(
                        "(i p) c -> p i c", p=128
                    ),
                )

                # mm1: res_c.T tiles [128H, 512L]; tanh(+rq bias) -> tmpT (fp16)
                tmpT_sb = tmpp.tile([128, HM, LCHUNK], F16, tag="tmpT")
                for m in range(HM):
                    ps_1 = ps_mm1.tile([128, LCHUNK], F32, tag="mm1")
                    for j in range(KC):
                        nc.tensor.matmul(
                            ps_1,
                            lhsT=wcT_sb[:, j, m * 128 : (m + 1) * 128],
                            rhs=ctxT_sb[:, j, :],
                            start=(j == 0),
                            stop=(j == KC - 1),
                        )
                    nc.scalar.activation(
                        tmpT_sb[:, m, :],
                        ps_1,
                        AF.Tanh,
                        bias=rq_sb[:, m * BLOC + b : m * BLOC + b + 1],
                        scale=1.0,
                    )

                # mm2: logits [1, 512] = sum_m wo_m.T @ tmpT_m
                ps_2 = ps_mm2.tile([1, LCHUNK], F32, tag="mm2")
                for m in range(HM):
                    nc.tensor.matmul(
                        ps_2,
                        lhsT=wo_sb[:, m : m + 1],
                        rhs=tmpT_sb[:, m, :],
                        start=(m == 0),
                        stop=(m == HM - 1),
                    )

                # wexp = mask * exp(logit + bo); also a fp16 copy for mm3 and
                # a per-chunk partial sum for the normalizer
                wseg = wrow_sb[0:1, c * LCHUNK : (c + 1) * LCHUNK]
                nc.scalar.activation(wseg, ps_2, AF.Exp, bias=bo_sb[0:1, 0:1], scale=1.0)
                nc.vector.tensor_mul(
                    wseg,
                    wseg,
                    mask_sb[0:1, b * L + c * LCHUNK : b * L + (c + 1) * LCHUNK],
                )
                nc.vector.tensor_reduce(
                    sparts_sb[0:1, c : c + 1],
                    wseg,
                    axis=mybir.AxisListType.X,
                    op=mybir.AluOpType.add,
                )

                # transpose wexp back to [L-part, 1] via K=1 matmuls with ones
                ps_w = ps_wt.tile([128, NSUB], F32, tag="wt")
                for i in range(NSUB):
                    nc.tensor.matmul(
                        ps_w[:, i : i + 1],
                        lhsT=wrow_sb[0:1, c * LCHUNK + i * 128 : c * LCHUNK + (i + 1) * 128],
                        rhs=onesf_sb[0:1, 0:1],
                        start=True,
                        stop=True,
                    )
                wT_sb = smallp.tile([128, NSUB], F16, tag="wT")
                nc.vector.tensor_copy(wT_sb, ps_w)

                # mm3: accumulate unnormalized output over the whole batch (fp16
                # operands, fp32 psum accumulation)
                for i in range(NSUB):
                    nc.tensor.matmul(
                        ps_o,
                        lhsT=wT_sb[:, i : i + 1],
                        rhs=ctx_sb[:, i, :],
                        start=(c == 0 and i == 0),
                        stop=(c == NCHUNKS - 1 and i == NSUB - 1),
                    )

            # ---- batch epilogue: normalize ----
            s_sb = smallp.tile([1, 1], F32, tag="s")
            nc.vector.tensor_reduce(
                s_sb, sparts_sb, axis=mybir.AxisListType.X, op=mybir.AluOpType.add
            )
            nc.vector.tensor_scalar_add(s_sb, s_sb, 1e-5)
            r_sb = smallp.tile([1, 1], F32, tag="r")
            nc.vector.reciprocal(r_sb, s_sb)

            wout_sb = wrowp.tile([1, L], F32, tag="wout")
            nc.vector.tensor_scalar_mul(wout_sb, wrow_sb, r_sb[0:1, 0:1])
            nc.sync.dma_start(w_d[b : b + 1, :], wout_sb)

            o_sb = smallp.tile([1, C], F32, tag="o")
            nc.vector.tensor_scalar_mul(o_sb, ps_o, r_sb[0:1, 0:1])
            nc.sync.dma_start(out_d[b : b + 1, :], o_sb)

    nc.compile()
    return nc


_NC_CACHE: list = []


def _get_nc() -> bass.Bass:
    if not _NC_CACHE:
        _NC_CACHE.append(build_nc())
    return _NC_CACHE[0]


def kernel(query, context, mapping_mask, Wq, Wc, bc, Wo, bo):
    query = np.asarray(query, dtype=np.float32)
    context = np.asarray(context, dtype=np.float32)
    mapping_mask = np.asarray(mapping_mask, dtype=np.float32)
    Wq = np.asarray(Wq, dtype=np.float32)
    Wc = np.asarray(Wc, dtype=np.float32)
    bc = np.asarray(bc, dtype=np.float32)
    Wo = np.asarray(Wo, dtype=np.float32)
    bo = np.asarray(bo, dtype=np.float32)

    wcT = np.ascontiguousarray(Wc.T).astype(np.float16)
    wqT = np.ascontiguousarray(Wq.T).astype(np.float16)
    wo = np.ascontiguousarray(Wo.reshape(HM, 128).T).astype(np.float16)
    bc2 = np.ascontiguousarray(bc.reshape(1, H))
    bo2 = np.ascontiguousarray(bo.reshape(1, 1))
    ctx16 = context.astype(np.float16)
    ctxT_all = np.ascontiguousarray(ctx16.transpose(0, 2, 1))

    in_maps = []
    for core in range(NCORES):
        sl = slice(core * BLOC, (core + 1) * BLOC)
        in_maps.append(
            {
                "ctx": np.ascontiguousarray(ctx16[sl]),
                "ctxT": np.ascontiguousarray(ctxT_all[sl]),
                "qT": np.ascontiguousarray(query[sl].T.astype(np.float16)),
                "mask": np.ascontiguousarray(mapping_mask[sl, :, 0]),
                "wcT": wcT,
                "wqT": wqT,
                "wo": wo,
                "bc": bc2,
                "bo": bo2,
            }
        )

    res = run_bass_kernel_spmd(_get_nc(), in_maps, core_ids=list(range(NCORES)))
    outs = res.results
    output = np.concatenate([o["out"] for o in outs], axis=0)
    weights = np.concatenate([o["w"] for o in outs], axis=0).reshape(B, L, 1)
    return output, weights


# revision 16
# speedup vs baseline: 3.6084x; 1.0526x over previous
"""Trainium2 Bass kernel for the additive-attention model.

Math (per batch b):
    res_q = query @ Wq.T                  [H]
    res_c = ctx @ Wc.T + bc               [L, H]
    tmp   = tanh(res_q + res_c)           [L, H]
    logit = tmp @ Wo.T + bo               [L, 1]
    wexp  = mask * exp(logit)             [L, 1]
    w     = wexp / (sum_L wexp + 1e-5)    [L, 1]
    out   = sum_L w * ctx                 [C]

Sharding: data-parallel over batch B=32 across 8 cores (4 batches/core),
weights replicated.  Everything is computed in a transposed layout
(H on partitions) so the res_q/bc bias folds into the tanh activation
and the Wo contraction is a matmul.  Matmul operands are fp16 (fp32
matmul on trn2 costs two PE passes, 16-bit one; fp16 keeps 10 mantissa
bits), with all accumulation/normalization in fp32.
"""

import sys

if "/opt/trn_rl_repo" not in sys.path:
    sys.path.insert(0, "/opt/trn_rl_repo")

from contextlib import ExitStack

import numpy as np

import concourse.bacc as bacc
import concourse.bass as bass
import concourse.tile as tile
from concourse import mybir
from concourse.bass_utils import run_bass_kernel_spmd

F32 = mybir.dt.float32
F16 = mybir.dt.float16
AF = mybir.ActivationFunctionType

B, L, C, Q, H = 32, 2048, 512, 512, 1024
NCORES = 8
BLOC = B // NCORES  # 4 batches per core
LCHUNK = 512
NCHUNKS = L // LCHUNK  # 4
NSUB = LCHUNK // 128  # 4 L-subtiles per chunk
HM = H // 128  # 8 H tiles (output partition dim of mm1)
KC = C // 128  # 4 contraction tiles over C
KQ = Q // 128  # 4 contraction tiles over Q


def build_nc() -> bass.Bass:
    nc = bacc.Bacc()

    ctx_d = nc.dram_tensor("ctx", (BLOC, L, C), F16, kind="ExternalInput")
    ctxT_d = nc.dram_tensor("ctxT", (BLOC, C, L), F16, kind="ExternalInput")
    qT_d = nc.dram_tensor("qT", (Q, BLOC), F16, kind="ExternalInput")
    mask_d = nc.dram_tensor("mask", (BLOC, L), F32, kind="ExternalInput")
    wcT_d = nc.dram_tensor("wcT", (C, H), F16, kind="ExternalInput")
    wqT_d = nc.dram_tensor("wqT", (Q, H), F16, kind="ExternalInput")
    wo_d = nc.dram_tensor("wo", (128, HM), F16, kind="ExternalInput")
    bc_d = nc.dram_tensor("bc", (1, H), F32, kind="ExternalInput")
    bo_d = nc.dram_tensor("bo", (1, 1), F32, kind="ExternalInput")

    out_d = nc.dram_tensor("out", (BLOC, C), F32, kind="ExternalOutput")
    w_d = nc.dram_tensor("w", (BLOC, L), F32, kind="ExternalOutput")

    with ExitStack() as ctx_es:
        tc = ctx_es.enter_context(tile.TileContext(nc))
        const = ctx_es.enter_context(tc.tile_pool(name="const", bufs=1))
        ctxp = ctx_es.enter_context(tc.tile_pool(name="ctxp", bufs=3))
        ctxTp = ctx_es.enter_context(tc.tile_pool(name="ctxTp", bufs=3))
        tmpp = ctx_es.enter_context(tc.tile_pool(name="tmpp", bufs=2))
        wrowp = ctx_es.enter_context(tc.tile_pool(name="wrowp", bufs=2))
        smallp = ctx_es.enter_context(tc.tile_pool(name="smallp", bufs=4))
        ps_mm1 = ctx_es.enter_context(tc.tile_pool(name="ps_mm1", bufs=2, space="PSUM"))
        ps_mm2 = ctx_es.enter_context(tc.tile_pool(name="ps_mm2", bufs=2, space="PSUM"))
        ps_wt = ctx_es.enter_context(tc.tile_pool(name="ps_wt", bufs=2, space="PSUM"))
        ps_out = ctx_es.enter_context(tc.tile_pool(name="ps_out", bufs=1, space="PSUM"))

        # ---- constants (wcT first: the first matmuls need it) ----
        wcT_sb = const.tile([128, KC, H], F16)
        nc.sync.dma_start(wcT_sb, wcT_d[:].rearrange("(k p) h -> p k h", p=128))
        wqT_sb = const.tile([128, KQ, H], F16)
        nc.sync.dma_start(wqT_sb, wqT_d[:].rearrange("(k p) h -> p k h", p=128))
        qT_sb = const.tile([128, KQ, BLOC], F16)
        nc.sync.dma_start(qT_sb, qT_d[:].rearrange("(k p) b -> p k b", p=128))
        wo_sb = const.tile([128, HM], F16)
        nc.sync.dma_start(wo_sb, wo_d[:])
        bc_sb = const.tile([1, H], F32)
        nc.sync.dma_start(bc_sb, bc_d[:])
        bo_sb = const.tile([1, 1], F32)
        nc.sync.dma_start(bo_sb, bo_d[:])
        # engine operands must start at partition 0, so keep the mask flat on one partition
        mask_sb = const.tile([1, BLOC * L], F32)
        nc.sync.dma_start(mask_sb, mask_d[:].rearrange("b l -> (b l)")[None, :])
        onesf_sb = const.tile([1, BLOC], F32)
        nc.vector.memset(onesf_sb, 1.0)
        ones16_sb = const.tile([1, 1], F16)
        nc.vector.memset(ones16_sb, 1.0)

        # ---- res_q + bc, transposed: rq_sb[p, m*BLOC + b] = (q_b @ Wq.T + bc)[m*128+p]
        ps_rq = ps_out.tile([128, HM * BLOC], F32, tag="rq")
        for m in range(HM):
            o = ps_rq[:, m * BLOC : (m + 1) * BLOC]
            for k in range(KQ):
                nc.tensor.matmul(
                    o,
                    lhsT=wqT_sb[:, k, m * 128 : (m + 1) * 128],
                    rhs=qT_sb[:, k, :],
                    start=(k == 0),
                    stop=False,
                )
            # + bc via rank-1 (K=1, fp32) matmul: bc_chunk.T @ ones
            nc.tensor.matmul(
                o,
                lhsT=bc_sb[0:1, m * 128 : (m + 1) * 128],
                rhs=onesf_sb[0:1, :],
                start=False,
                stop=True,
            )
        rq_sb = const.tile([128, HM * BLOC], F32)
        nc.vector.tensor_copy(rq_sb, ps_rq)

        # ---- main loop ----
        for b in range(BLOC):
            wrow_sb = wrowp.tile([1, L], F32, tag="wrow")
            sparts_sb = smallp.tile([1, NCHUNKS], F32, tag="sparts")
            ps_o = ps_out.tile([1, C], F32, tag="out")
            for c in range(NCHUNKS):
                # ctx chunk transposed (fp16, rhs of the res_c matmul) -- load
                # first, mm1 consumes it immediately
                ctxT_sb = ctxTp.tile([128, KC, LCHUNK], F16, tag="ctxT")
                nc.sync.dma_start(
                    ctxT_sb,
                    ctxT_d[b, :, c * LCHUNK : (c + 1) * LCHUNK].rearrange(
                        "(j p) l -> p j l", p=128
                    ),
                )
                # ctx chunk in natural layout (fp16, for the output matmul)
                ctx_sb = ctxp.tile([128, NSUB, C], F16, tag="ctx")
                nc.sync.dma_start(
                    ctx_sb,
                    ctx_d[b, c * LCHUNK : (c + 1) * LCHUNK, :].rearrange(
                        "(i p) c -> p i c", p=128
                    ),
                )

                # mm1: res_c.T tiles [128H, 512L]; tanh(+rq bias) -> tmpT (fp16)
                tmpT_sb = tmpp.tile([128, HM, LCHUNK], F16, tag="tmpT")
                for m in range(HM):
                    ps_1 = ps_mm1.tile([128, LCHUNK], F32, tag="mm1")
                    for j in range(KC):
                        nc.tensor.matmul(
                            ps_1,
                            lhsT=wcT_sb[:, j, m * 128 : (m + 1) * 128],
                            rhs=ctxT_sb[:, j, :],
                            start=(j == 0),
                            stop=(j == KC - 1),
                        )
                    nc.scalar.activation(
                        tmpT_sb[:, m, :],
                        ps_1,
                        AF.Tanh,
                        bias=rq_sb[:, m * BLOC + b : m * BLOC + b + 1],
                        scale=1.0,
                    )

                # mm2: logits [1, 512] = sum_m wo_m.T @ tmpT_m
                ps_2 = ps_mm2.tile([1, LCHUNK], F32, tag="mm2")
                for m in range(HM):
                    nc.tensor.matmul(
                        ps_2,
                        lhsT=wo_sb[:, m : m + 1],
                        rhs=tmpT_sb[:, m, :],
                        start=(m == 0),
                        stop=(m == HM - 1),
                    )

                # wexp = mask * exp(logit + bo); also a fp16 copy for mm3 and
                # a per-chunk partial sum for the normalizer
                wseg = wrow_sb[0:1, c * LCHUNK : (c + 1) * LCHUNK]
                nc.scalar.activation(wseg, ps_2, AF.Exp, bias=bo_sb[0:1, 0:1], scale=1.0)
                nc.vector.tensor_mul(
                    wseg,
                    wseg,
                    mask_sb[0:1, b * L + c * LCHUNK : b * L + (c + 1) * LCHUNK],
                )
                nc.vector.tensor_reduce(
                    sparts_sb[0:1, c : c + 1],
                    wseg,
                    axis=mybir.AxisListType.X,
                    op=mybir.AluOpType.add,
                )

                # transpose wexp back to [L-part, 1] via K=1 matmuls with ones
                # (fp16 single-pass; fp32 matmuls cost two PE passes)
                wseg16 = smallp.tile([1, LCHUNK], F16, tag="wseg16")
                nc.vector.tensor_copy(wseg16, wseg)
                ps_w = ps_wt.tile([128, NSUB], F32, tag="wt")
                for i in range(NSUB):
                    nc.tensor.matmul(
                        ps_w[:, i : i + 1],
                        lhsT=wseg16[0:1, i * 128 : (i + 1) * 128],
                        rhs=ones16_sb[0:1, 0:1],
                        start=True,
                        stop=True,
                    )
                wT_sb = smallp.tile([128, NSUB], F16, tag="wT")
                nc.vector.tensor_copy(wT_sb, ps_w)

                # mm3: accumulate unnormalized output over the whole batch (fp16
                # operands, fp32 psum accumulation)
                for i in range(NSUB):
                    nc.tensor.matmul(
                        ps_o,
                        lhsT=wT_sb[:, i : i + 1],
                        rhs=ctx_sb[:, i, :],
                        start=(c == 0 and i == 0),
                        stop=(c == NCHUNKS - 1 and i == NSUB - 1),
                    )

            # ---- batch epilogue: normalize ----
            s_sb = smallp.tile([1, 1], F32, tag="s")
            nc.vector.tensor_reduce(
                s_sb, sparts_sb, axis=mybir.AxisListType.X, op=mybir.AluOpType.add
            )
            nc.vector.tensor_scalar_add(s_sb, s_sb, 1e-5)
            r_sb = smallp.tile([1, 1], F32, tag="r")
            nc.vector.reciprocal(r_sb, s_sb)

            wout_sb = wrowp.tile([1, L], F32, tag="wout")
            nc.vector.tensor_scalar_mul(wout_sb, wrow_sb, r_sb[0:1, 0:1])
            nc.sync.dma_start(w_d[b : b + 1, :], wout_sb)

            o_sb = smallp.tile([1, C], F32, tag="o")
            nc.vector.tensor_scalar_mul(o_sb, ps_o, r_sb[0:1, 0:1])
            nc.sync.dma_start(out_d[b : b + 1, :], o_sb)

    nc.compile()
    return nc


_NC_CACHE: list = []


def _get_nc() -> bass.Bass:
    if not _NC_CACHE:
        _NC_CACHE.append(build_nc())
    return _NC_CACHE[0]


def kernel(query, context, mapping_mask, Wq, Wc, bc, Wo, bo):
    query = np.asarray(query, dtype=np.float32)
    context = np.asarray(context, dtype=np.float32)
    mapping_mask = np.asarray(mapping_mask, dtype=np.float32)
    Wq = np.asarray(Wq, dtype=np.float32)
    Wc = np.asarray(Wc, dtype=np.float32)
    bc = np.asarray(bc, dtype=np.float32)
    Wo = np.asarray(Wo, dtype=np.float32)
    bo = np.asarray(bo, dtype=np.float32)

    wcT = np.ascontiguousarray(Wc.T).astype(np.float16)
    wqT = np.ascontiguousarray(Wq.T).astype(np.float16)
    wo = np.ascontiguousarray(Wo.reshape(HM, 128).T).astype(np.float16)
    bc2 = np.ascontiguousarray(bc.reshape(1, H))
    bo2 = np.ascontiguousarray(bo.reshape(1, 1))
    ctx16 = context.astype(np.float16)
    ctxT_all = np.ascontiguousarray(ctx16.transpose(0, 2, 1))

    in_maps = []
    for core in range(NCORES):
        sl = slice(core * BLOC, (core + 1) * BLOC)
        in_maps.append(
            {
                "ctx": np.ascontiguousarray(ctx16[sl]),
                "ctxT": np.ascontiguousarray(ctxT_all[sl]),
                "qT": np.ascontiguousarray(query[sl].T.astype(np.float16)),
                "mask": np.ascontiguousarray(mapping_mask[sl, :, 0]),
                "wcT": wcT,
                "wqT": wqT,
                "wo": wo,
                "bc": bc2,
                "bo": bo2,
            }
        )

    res = run_bass_kernel_spmd(_get_nc(), in_maps, core_ids=list(range(NCORES)))
    outs = res.results
    output = np.concatenate([o["out"] for o in outs], axis=0)
    weights = np.concatenate([o["w"] for o in outs], axis=0).reshape(B, L, 1)
    return output, weights
